# revision 13
# baseline (speedup 1.0000x reference)
"""Trainium2 Bass kernel for nn_BendingDiffSort_XY.

Data-parallel over batch B=32 across 8 NeuronCores (4 batches/core).
Pipeline per batch:
  conv1/conv2 (fp32 matmuls, channel-partition layout) -> relu (ACT/DVE)
  row/col scores: DVE multiply + in-place segmented tree reduce + PE ones-matmul
  bitonic diffsort: 21 layers, 2 stacks of 4 chains, per layer one fp32
    (I - XORperm) matmul producing D = Q - Qshuf, ACT arctan for alpha,
    DVE scalar_tensor_tensor update  Q += (alpha-1) * D   (x column included)
  bmm: P_row/P_col applied as bf16 matmuls batched over channels with a
    DMA-xbar transpose (bf16) between them.
Precision: conv/score path fp32 (bf16/fp16 fail: steepness-50 arctan
amplifies near-tie score errors), bmm path bf16 (~6e-3 rel).
"""

import numpy as np
import ml_dtypes

B, C, N = 32, 128, 64
HID = 2 * C
STEEP = 50.0
NB = 4            # batches per core
NCORES = 8
SP = N * N        # 4096 spatial
NL = 21           # bitonic layers

F32 = None  # set lazily (mybir import inside kernel)


def _bitonic_layers(n):
    num_blocks = int(np.log2(n))
    layers = []
    for block in range(num_blocks):
        for layer in range(block + 1):
            m = 2 ** (block - layer)
            a_idx, b_idx = [], []
            for i in range(0, n, 2 * m):
                for j in range(m):
                    ix = i + j
                    a, b = ix, ix + m
                    if (ix // 2 ** (block + 1)) % 2 == 1:
                        a, b = b, a
                    a_idx.append(a)
                    b_idx.append(b)
            layers.append((np.asarray(a_idx), np.asarray(b_idx), m))
    return layers


def _host_consts():
    layers = _bitonic_layers(N)
    # sigma per layer: +1 on 'a' slots, -1 on 'b' slots; ACT scale = -STEEP*sigma
    sig = np.zeros((N, NL), np.float32)
    midx = []
    dist_m = sorted({m for _, _, m in layers})
    for t, (a_idx, b_idx, m) in enumerate(layers):
        sig[a_idx, t] = 1.0
        sig[b_idx, t] = -1.0
        midx.append(dist_m.index(m))
    sig_t = np.vstack([sig, sig]) * (-STEEP)          # [128, 21]
    ixm = np.zeros((len(dist_m), 2 * N, 2 * N), np.float32)
    for k, m in enumerate(dist_m):
        X = np.zeros((N, N), np.float32)
        for p in range(N):
            X[p, p ^ m] = 1.0
        IX = np.eye(N, dtype=np.float32) - X
        ixm[k][:N, :N] = IX
        ixm[k][N:, N:] = IX
    qx0 = np.zeros((2 * N, 2 * 65), np.float32)       # [128, 130]
    for q in range(2):
        qx0[:N, 65 * q:65 * q + N] = np.eye(N)
        qx0[N:, 65 * q:65 * q + N] = np.eye(N)
    return sig_t, midx, ixm, qx0


def build(tc, outs, ins):
    import concourse.bass as bass
    import concourse.mybir as mybir
    from contextlib import ExitStack

    nc = tc.nc
    f32 = mybir.dt.float32
    bf16 = mybir.dt.bfloat16
    AF = mybir.ActivationFunctionType
    OP = mybir.AluOpType

    x_d = ins["x4"]            # [4, 128, 4096] f32
    w1T_d = ins["w1T"]         # [128, 256] f32
    w2T_d = ins["w2T"]         # [2, 128, 256] f32
    b1_d = ins["b1t"]          # [128, 2]
    b2_d = ins["b2t"]
    wrr_d = ins["wrr"]         # [2, 128, 64] row weights per c-tile
    wrc_d = ins["wrc"]
    brf_d = ins["brf"]         # [128, 2] col0 = b_row/128, col1 = b_col/128
    ones_d = ins["ones1"]      # [128, 1]
    sig_d = ins["sig"]         # [128, 21]
    ixm_d = ins["ixm"]         # [6, 128, 128]
    qx0_d = ins["qx0"]         # [128, 130]
    out_d = outs["out"]        # [4, 128, 4096] f32

    layers = _bitonic_layers(N)
    dist_m = sorted({m for _, _, m in layers})
    midx = [dist_m.index(m) for _, _, m in layers]

    with ExitStack() as ctx:
        cpool = ctx.enter_context(tc.tile_pool(name="consts", bufs=1))
        xpool = ctx.enter_context(tc.tile_pool(name="x", bufs=9))
        hpool = ctx.enter_context(tc.tile_pool(name="h", bufs=1))
        spool = ctx.enter_context(tc.tile_pool(name="sc", bufs=2))
        qpool = ctx.enter_context(tc.tile_pool(name="q", bufs=1))
        mpool = ctx.enter_context(tc.tile_pool(name="mm", bufs=1))
        opool = ctx.enter_context(tc.tile_pool(name="ost", bufs=4))
        pps = ctx.enter_context(tc.tile_pool(name="ps", bufs=4, space="PSUM"))
        pps2 = ctx.enter_context(tc.tile_pool(name="ps2", bufs=2, space="PSUM"))
        pps3 = ctx.enter_context(tc.tile_pool(name="ps3", bufs=2, space="PSUM"))

        # ---- persistent constants ----
        w1T = cpool.tile([128, 256], f32, tag="w1T")
        nc.sync.dma_start(w1T[:], w1T_d[:])
        w2T = [cpool.tile([128, 256], f32, tag=f"w2T{k}", name=f"w2T{k}") for k in range(2)]
        for k in range(2):
            nc.sync.dma_start(w2T[k][:], w2T_d[k])
        b1t = cpool.tile([128, 2], f32, tag="b1t")
        nc.sync.dma_start(b1t[:], b1_d[:])
        b2t = cpool.tile([128, 2], f32, tag="b2t")
        nc.sync.dma_start(b2t[:], b2_d[:])
        wrr = cpool.tile([128, 2, 64], f32, tag="wrr")
        nc.sync.dma_start(wrr[:], wrr_d.rearrange("t p w -> p t w"))
        wrc = cpool.tile([128, 2, 64], f32, tag="wrc")
        nc.sync.dma_start(wrc[:], wrc_d.rearrange("t p w -> p t w"))
        brf = cpool.tile([128, 2], f32, tag="brf")
        nc.sync.dma_start(brf[:], brf_d[:])
        ones1 = cpool.tile([128, 1], f32, tag="ones1")
        nc.sync.dma_start(ones1[:], ones_d[:])
        sig = cpool.tile([128, 21], f32, tag="sig")
        nc.sync.dma_start(sig[:], sig_d[:])
        ixm = [cpool.tile([128, 128], f32, tag=f"ixm{k}", name=f"ixm{k}") for k in range(6)]
        for k in range(6):
            nc.sync.dma_start(ixm[k][:], ixm_d[k])

        # sort stacks (one per batch pair), alive across phases
        QX = [qpool.tile([128, 130], f32, tag=f"qx{s}", name=f"qx{s}") for s in range(2)]

        def conv_and_scores(b):
            s, half = b // 2, b % 2
            if "conv" in _ABLATE:
                return
            h1 = [hpool.tile([128, SP], f32, tag=f"h1_{ct}", name=f"h1_{ct}") for ct in range(2)]
            for j in range(8):
                xch = xpool.tile([128, 512], f32, tag="xch")
                nc.sync.dma_start(xch[:], x_d[b, :, 512 * j:512 * (j + 1)])
                for ot in range(2):
                    ps = pps.tile([128, 512], f32, tag="ps")
                    nc.tensor.matmul(ps[:], w1T[:, 128 * ot:128 * (ot + 1)],
                                     xch[:], start=True, stop=True)
                    dst = h1[ot][:, 512 * j:512 * (j + 1)]
                    if ot == 0:
                        nc.scalar.activation(dst, ps[:], AF.Relu,
                                             bias=b1t[:, 0:1], scale=1.0)
                    else:
                        nc.vector.tensor_scalar(dst, ps[:], b1t[:, 1:2], 0.0,
                                                op0=OP.add, op1=OP.max)
            h2 = [hpool.tile([128, SP], f32, tag=f"h2_{ct}", name=f"h2_{ct}") for ct in range(2)]
            for ot in range(2):
                for j in range(8):
                    ps = pps.tile([128, 512], f32, tag="ps")
                    nc.tensor.matmul(ps[:], w2T[0][:, 128 * ot:128 * (ot + 1)],
                                     h1[0][:, 512 * j:512 * (j + 1)],
                                     start=True, stop=False)
                    nc.tensor.matmul(ps[:], w2T[1][:, 128 * ot:128 * (ot + 1)],
                                     h1[1][:, 512 * j:512 * (j + 1)],
                                     start=False, stop=True)
                    dst = h2[ot][:, 512 * j:512 * (j + 1)]
                    if ot == 0:
                        nc.scalar.activation(dst, ps[:], AF.Relu,
                                             bias=b2t[:, 0:1], scale=1.0)
                    else:
                        nc.vector.tensor_scalar(dst, ps[:], b2t[:, 1:2], 0.0,
                                                op0=OP.add, op1=OP.max)
            # scores: branch 0 = row (reduce over w), branch 1 = col (reduce over h)
            if "scores" in _ABLATE:
                return
            for br, wt in ((0, wrr), (1, wrc)):
                rts = []
                for ct in range(2):
                    t = spool.tile([128, 64, 64], f32, tag="sct", name=f"sct")
                    h2v = h2[ct][:, :].rearrange("p (h w) -> p h w", h=64)
                    if br == 0:
                        wb = wt[:, ct, :].broadcast_to([128, 64, 64]).rearrange("p w h -> p h w")
                    else:
                        wb = wt[:, ct, :].broadcast_to([128, 64, 64])
                    nc.vector.tensor_mul(t[:], h2v, wb)
                    # in-place tree reduce over w (br0) or h (br1)
                    wdim = 64
                    while wdim > 1:
                        hw = wdim // 2
                        if br == 0:
                            nc.vector.tensor_add(t[:, :, 0:hw], t[:, :, 0:hw],
                                           t[:, :, hw:wdim])
                        else:
                            nc.vector.tensor_add(t[:, 0:hw, :], t[:, 0:hw, :],
                                           t[:, hw:wdim, :])
                        wdim = hw
                    rts.append(t)
                rt = spool.tile([128, 64], f32, tag="rt")
                if br == 0:
                    v0 = rts[0][:, :, 0:1].rearrange("p h o -> p (h o)")
                    v1 = rts[1][:, :, 0:1].rearrange("p h o -> p (h o)")
                else:
                    v0 = rts[0][:, 0:1, :].rearrange("p o w -> p (o w)")
                    v1 = rts[1][:, 0:1, :].rearrange("p o w -> p (o w)")
                nc.vector.scalar_tensor_tensor(rt[:], v0, brf[:, br:br + 1], v1,
                                               op0=OP.add, op1=OP.add)
                ps = pps3.tile([128, 1], f32, tag="scps")
                if half == 0:
                    nc.tensor.matmul(ps[0:64, :], rt[:], ones1[:],
                                     start=True, stop=True)
                    nc.vector.tensor_copy(QX[s][0:64, 65 * br + 64:65 * br + 65],
                                          ps[0:64, :])
                else:
                    nc.tensor.matmul(ps[64:128, :], rt[:], ones1[:],
                                     start=True, stop=True, tile_position=(0, 64))
                    nc.vector.tensor_copy(QX[s][64:128, 65 * br + 64:65 * br + 65],
                                          ps[64:128, :])

        def sort_stack(s):
            if "sort" in _ABLATE:
                return
            for t in range(NL):
                psd = pps2.tile([128, 130], f32, tag="srt")
                nc.tensor.matmul(psd[:], ixm[midx[t]][:], QX[s][:, :],
                                 start=True, stop=True)
                aat = spool.tile([128, 2], f32, tag="aat")
                dxv = psd[:, :].rearrange("p (q c) -> p q c", c=65)[:, :, 64:65].rearrange("p q o -> p (q o)")
                nc.scalar.activation(aat[:], dxv, AF.Arctan,
                                     bias=0.0, scale=sig[:, t:t + 1])
                am1 = spool.tile([128, 2], f32, tag="am1")
                nc.vector.tensor_scalar(am1[:], aat[:], float(1.0 / np.pi), -0.5,
                                        op0=OP.mult, op1=OP.add)
                for q in range(2):
                    sl = slice(65 * q, 65 * q + 65)
                    nc.vector.scalar_tensor_tensor(
                        QX[s][:, sl], psd[:, sl], am1[:, q:q + 1], QX[s][:, sl],
                        op0=OP.mult, op1=OP.add)

        def bmm_group(s):
            if "bmm" in _ABLATE:
                return
            bf = mpool
            # block-diag lhsT for mm1 (row perm), per group
            qrow = bf.tile([128, 128], bf16, tag="qrow")
            nc.gpsimd.memset(qrow[:], 0.0)
            nc.vector.tensor_copy(qrow[0:64, 0:64], QX[s][0:64, 0:64])
            nc.vector.tensor_copy(qrow[64:128, 64:128], QX[s][64:128, 0:64])
            # per batch lhsT for mm2 (col perm)
            qcol = []
            for b2 in range(2):
                qc = bf.tile([128, 128], bf16, tag=f"qcol{b2}", name=f"qcol{b2}")
                nc.gpsimd.memset(qc[:], 0.0)
                src = QX[s][64 * b2:64 * b2 + 64, 65:129]
                nc.vector.tensor_copy(qc[64 * b2:64 * b2 + 64,
                                         64 * b2:64 * b2 + 64], src)
                od = 64 * (1 - b2)
                nc.gpsimd.dma_start(qc[od:od + 64, od:od + 64], src)
                qcol.append(qc)
            # mm1 rhs: x in [(2b h), (c w)] bf16, loaded from DRAM with cast
            xh = bf.tile([128, 8192], bf16, tag="xh")
            for b2 in range(2):
                src = x_d[2 * s + b2].rearrange("c (h w) -> h c w", h=64)
                nc.gpsimd.dma_start(xh[64 * b2:64 * b2 + 64, :].rearrange("p (c w) -> p c w", c=128), src)
            x1 = bf.tile([128, 8192], bf16, tag="x1")
            if "mm1" in _ABLATE:
                return
            for j in range(16):
                ps = pps.tile([128, 512], f32, tag="ps")
                nc.tensor.matmul(ps[:], qrow[:], xh[:, 512 * j:512 * (j + 1)],
                                 start=True, stop=True)
                dst = x1[:, 512 * j:512 * (j + 1)]
                if j % 2 == 0:
                    nc.scalar.activation(dst, ps[:], AF.Copy, bias=0.0, scale=1.0)
                else:
                    nc.vector.tensor_copy(dst, ps[:])
            for b2 in range(2):
                b = 2 * s + b2
                x1t = bf.tile([128, 64, 64], bf16, tag=f"x1t{b2}", name=f"x1t{b2}")
                if "xbar" not in _ABLATE:
                    nc.sync.dma_start(x1t[:], x1[64 * b2:64 * b2 + 64, :],
                                      transpose=True)
                if "bmm2" in _ABLATE:
                    continue
                ov = out_d[b].rearrange("(cp two) (i k) -> (two i) cp k", two=2, k=64)
                for j in range(8):
                    ps = pps.tile([128, 512], f32, tag="ps")
                    nc.tensor.matmul(ps[:], qcol[b2][:],
                                     x1t[:, 8 * j:8 * (j + 1), :],
                                     start=True, stop=True)
                    ob = opool.tile([128, 8, 64], f32, tag="ob")
                    if j % 2 == 0:
                        nc.scalar.activation(ob[:], ps[:].rearrange("p (a k) -> p a k", a=8),
                                             AF.Copy, bias=0.0, scale=1.0)
                    else:
                        nc.vector.tensor_copy(ob[:], ps[:].rearrange("p (a k) -> p a k", a=8))
                    if "store" not in _ABLATE:
                        nc.sync.dma_start(ov[:, 8 * j:8 * (j + 1), :], ob[:])

        nc.sync.dma_start(QX[0][:, :], qx0_d[:])
        nc.sync.dma_start(QX[1][:, :], qx0_d[:])
        for b in range(2):
            conv_and_scores(b)
        sort_stack(0)
        for b in range(2, 4):
            conv_and_scores(b)
        bmm_group(0)
        sort_stack(1)
        bmm_group(1)


_CACHE = {}
_ABLATE = set()


def _compile():
    key = tuple(sorted(_ABLATE))
    if key in _CACHE:
        return _CACHE[key]
    from concourse import bacc
    import concourse.tile as tile
    import concourse.mybir as mybir

    f32 = mybir.dt.float32
    nc = bacc.Bacc("TRN2", target_bir_lowering=False, debug=False)
    ins = {
        "x4": nc.dram_tensor("x4", [NB, C, SP], f32, kind="ExternalInput").ap(),
        "w1T": nc.dram_tensor("w1T", [C, HID], f32, kind="ExternalInput").ap(),
        "w2T": nc.dram_tensor("w2T", [2, C, HID], f32, kind="ExternalInput").ap(),
        "b1t": nc.dram_tensor("b1t", [C, 2], f32, kind="ExternalInput").ap(),
        "b2t": nc.dram_tensor("b2t", [C, 2], f32, kind="ExternalInput").ap(),
        "wrr": nc.dram_tensor("wrr", [2, C, N], f32, kind="ExternalInput").ap(),
        "wrc": nc.dram_tensor("wrc", [2, C, N], f32, kind="ExternalInput").ap(),
        "brf": nc.dram_tensor("brf", [C, 2], f32, kind="ExternalInput").ap(),
        "ones1": nc.dram_tensor("ones1", [C, 1], f32, kind="ExternalInput").ap(),
        "sig": nc.dram_tensor("sig", [C, NL], f32, kind="ExternalInput").ap(),
        "ixm": nc.dram_tensor("ixm", [6, C, C], f32, kind="ExternalInput").ap(),
        "qx0": nc.dram_tensor("qx0", [C, 130], f32, kind="ExternalInput").ap(),
    }
    outs = {"out": nc.dram_tensor("out", [NB, C, SP], f32,
                                  kind="ExternalOutput").ap()}
    with tile.TileContext(nc) as tc:
        build(tc, outs, ins)
    nc.compile()
    _CACHE[key] = nc
    return nc


def _in_maps(inputs):
    x = np.ascontiguousarray(inputs["x"], np.float32)
    sig_t, midx, ixm, qx0 = _host_consts()
    common = {
        "w1T": np.ascontiguousarray(inputs["w1"].T, np.float32),
        "w2T": np.ascontiguousarray(
            inputs["w2"].T.reshape(2, C, HID), np.float32),
        "b1t": np.ascontiguousarray(
            inputs["b1"].reshape(2, C).T, np.float32),
        "b2t": np.ascontiguousarray(
            inputs["b2"].reshape(2, C).T, np.float32),
        "wrr": np.ascontiguousarray(
            inputs["w_row"].reshape(2, C, N), np.float32),
        "wrc": np.ascontiguousarray(
            inputs["w_col"].reshape(2, C, N), np.float32),
        "brf": np.ascontiguousarray(np.stack(
            [np.full(C, inputs["b_row"][0] / C),
             np.full(C, inputs["b_col"][0] / C)], axis=1), np.float32),
        "ones1": np.ones((C, 1), np.float32),
        "sig": np.ascontiguousarray(sig_t, np.float32),
        "ixm": np.ascontiguousarray(ixm, np.float32),
        "qx0": np.ascontiguousarray(qx0, np.float32),
    }
    maps = []
    for k in range(NCORES):
        m = dict(common)
        m["x4"] = np.ascontiguousarray(
            x[NB * k:NB * (k + 1)].reshape(NB, C, SP), np.float32)
        maps.append(m)
    return maps


def run(inputs, trace=False):
    from concourse import bass_utils
    nc = _compile()
    res = bass_utils.run_bass_kernel_spmd(
        nc, _in_maps(inputs), core_ids=list(range(NCORES)), trace=trace)
    out = np.concatenate([r["out"] for r in res.results], axis=0)
    return out.reshape(B, C, N, N).astype(np.float32), res


def kernel(**inputs):
    out, _ = run(inputs, trace=False)
    return out
